# revision 20
# baseline (speedup 1.0000x reference)
"""LIF spiking-neuron kernel for Trainium2 (Bass/Tile), 8-core data-parallel.

Reference semantics (per element, scan over T=8):
    mem = mem * 0.5 + x_t
    s_t = (mem > 1.0) ? 1.0 : 0.0        # forward value of the spike
    mem = mem - s_t

Spikes leave the device as ONE byte per element: the ACT engine writes
sg_t = fp8e4(Sign(mem_t - thr)) in a single op ({-1,0,+1}; no Relu
needed) and the host decodes spikes as (byte == 0x38), the fp8 encoding
of +1.0. This cuts HBM store traffic 4x vs fp32 (16.78 MB in + 4.19 MB
out per core) and puts the input stream, at ~340 GB/s effective, level
with the compute.

The free axis (4096 fp32/partition after folding batch*C*H*W) is cut
into four 1024-wide chunks that all advance together through the T=8
scan, so each x_t tile is consumed in load order. Per global step the
chunks split across two engine pipelines, sized so the Vector engine
(the throughput limiter at ~123 G elem/s) tracks the DMA load rate:

  PE-assisted pair (cols 0:2048, membrane in PSUM, 1 DVE op/step):
      ACT:  sg_t = fp8(Sign(p_t - (1+c_t)))       # also the output tile
      DVE:  q_{t+1} = 0.5 * p_t + x_{t+1}         # stt into a PSUM bank
      PE:   q_{t+1} += (-0.25 I) @ sg_t           # fp8 matmul-accumulate
  where p_t = mem_t + c_t, c_{t+1} = 0.5*c_t + 0.25 (exact dyadic): the
  reset -0.5*s_t = -0.25*sg_t - 0.25 splits into the PE matmul plus a
  constant folded into the threshold. The accumulate onto the
  DVE-written bank is exact but the dep tracker sees two independent
  writers, so the DVE->PE (and PE->reader) edges are added explicitly.
  PSUM fits exactly two such chunks' ping-pong state (8 banks).

  DVE-only pair (cols 2048:4096, carry neg_mem = -mem in SBUF):
      m_t       = stt(neg_mem, -0.5, x_t, mult, add)
      sg_t      = fp8(Sign(m_t - 1))              # ACT, output tile
      neg_mem_t = stt(m_t, 1.0, m_t, is_gt, subtract)   # (m>1) - m

Each x half-tile has exactly one DMA writer (two DMAs into one tile
defeat the subtile dep tracker) and loads/stores alternate between the
SP and Pool DGE queues; DVE, ACT and PE issue no DMAs.

Sharding: batch dim B=32 (dim 1 after temporal expand) split across 8
cores, 4 per core -> per-core [T=8, 128 partitions, 4096 free] fp32.
"""

import numpy as np
import ml_dtypes

import concourse.bass as bass
import concourse.bacc as bacc
import concourse.tile as tile
from concourse import mybir
from concourse.bass_utils import run_bass_kernel_spmd
from concourse.tile_rust import add_dep_helper

T = 8
B = 32
C = 128
H = 32
W = 32
NCORES = 8
BL = B // NCORES              # 4 batch elements per core
N = BL * C * H * W            # 524288 elements per timestep per core
P = 128                       # SBUF partitions
FREE = N // P                 # 4096 fp32 per partition per timestep
FCHUNK = 1024                 # compute chunk width (2 PSUM banks fp32)
LCHUNK = 2048                 # load chunk (per-dma granularity)
MMF = 512                     # matmul moving free dim / PSUM bank width

_ALU = mybir.AluOpType

# Offset-membrane constants: c_0 = 0, c_{t+1} = 0.5*c_t + 0.25 (dyadic).
_CS = [0.0]
for _ in range(T - 1):
    _CS.append(0.5 * _CS[-1] + 0.25)
_THR = [-(1.0 + c) for c in _CS]   # ACT bias per step, PE-assisted pair


def build_bass(free: int = FREE):
    nc = bacc.Bacc("TRN2", target_bir_lowering=False, debug=False,
                   num_devices=NCORES)
    x_ap = nc.dram_tensor("x", [T, P, free], mybir.dt.float32,
                          kind="ExternalInput").ap()
    w_ap = nc.dram_tensor("w", [P, P], mybir.dt.float8e4,
                          kind="ExternalInput").ap()
    o_ap = nc.dram_tensor("out", [T, P, free], mybir.dt.float8e4,
                          kind="ExternalOutput").ap()

    nhalf = free // LCHUNK
    _F = mybir.ActivationFunctionType
    with tile.TileContext(nc) as tc:
        with (
            tc.tile_pool(name="xp", bufs=T) as xp,
            tc.tile_pool(name="xqp", bufs=2) as xqp,
            tc.tile_pool(name="sgp", bufs=6) as sgp,
            tc.tile_pool(name="mp", bufs=2) as mp,
            tc.psum_pool(name="qp", bufs=4) as qp,
            tc.tile_pool(name="cp", bufs=1) as cp,
        ):
            wt = cp.tile([P, P], mybir.dt.float8e4, tag="w")
            nc.gpsimd.dma_start(wt[:], w_ap)
            biases = []
            for t in range(T):
                bt = cp.tile([P, 1], mybir.dt.float32, tag=f"b{t}")
                nc.gpsimd.memset(bt[:], _THR[t])
                biases.append(bt)

            # Preload all of x in consumption order. t=0/1 use quarter
            # tiles for the PE-pair columns so the very first DVE/ACT ops
            # have data ~8us earlier; the rest are half tiles. Each tile
            # has exactly ONE DMA writer (the subtile dep tracker cannot
            # handle two), units alternate between the two DGE queues.
            units = {}   # (t, lo) -> (tile, width)
            qi = 0

            def load_unit(t, lo, width, pool, tag):
                nonlocal qi
                xt = pool.tile([P, width], mybir.dt.float32, tag=tag)
                eng = nc.sync if qi % 2 == 0 else nc.gpsimd
                qi += 1
                eng.dma_start(xt[:], x_ap[t, :, bass.ds(lo, width)])
                units[(t, lo)] = (xt, width)

            for t in (0, 1):
                load_unit(t, 0, FCHUNK, xqp, "xq0")
                load_unit(t, FCHUNK, FCHUNK, xqp, "xq1")
            load_unit(0, LCHUNK, LCHUNK, xp, "xh1")
            load_unit(1, LCHUNK, LCHUNK, xp, "xh1")
            for t in range(2, T):
                load_unit(t, 0, LCHUNK, xp, "xh0")
                load_unit(t, LCHUNK, LCHUNK, xp, "xh1")

            def xsl(t, lo, width):
                # [128, width] view of cols [lo, lo+width) of x_t
                for (ut, ulo), (xt, uw) in units.items():
                    if ut == t and ulo <= lo and lo + width <= ulo + uw:
                        return xt[:, bass.ds(lo - ulo, width)]
                raise KeyError((t, lo, width))

            pe_chunks = (0, 1)
            q = {c: None for c in pe_chunks}       # PSUM membrane, PE pair
            mms = {c: [] for c in pe_chunks}       # last accum matmuls
            neg_mem = None                         # SBUF carry, DVE chunk

            for t in range(T):
                # One wide spike tile per step: the chunk Sign ops write
                # disjoint slices and a single coarse DMA stores it,
                # keeping the DGE rings dense (32 small stores would gate
                # the ring heads and idle the DMA engines).
                sgall = sgp.tile([P, free], mybir.dt.float8e4, tag="sg")
                # --- PE-assisted pair: cols [0 : 2*FCHUNK) ---
                for c in pe_chunks:
                    p_t = xsl(0, c * FCHUNK, FCHUNK) if t == 0 else q[c][:]
                    sg = sgall[:, bass.ts(c, FCHUNK)]
                    dve = None
                    if t < T - 1:
                        qn = qp.tile([P, FCHUNK], mybir.dt.float32, tag="q")
                        dve = nc.vector.scalar_tensor_tensor(
                            qn[:], p_t, 0.5, xsl(t + 1, c * FCHUNK, FCHUNK),
                            _ALU.mult, _ALU.add)
                        for m in mms[c]:
                            add_dep_helper(dve.ins, m,
                                           reason="q update reads accum q")
                    act = nc.scalar.activation(sg, p_t, _F.Sign,
                                               bias=biases[t][:])
                    for m in mms[c]:
                        add_dep_helper(act.ins, m,
                                       reason="spike reads accum q")
                    if dve is not None:
                        # The matmul below accumulates onto the DVE-written
                        # bank. Its semaphore wait alone can land inside the
                        # DVE->PSUM write-drain window; ordering ACT after the
                        # stt puts the (sg-dependent) matmul >1us behind the
                        # write, closing the race.
                        add_dep_helper(act.ins, dve.ins,
                                       reason="order spike after q write")
                        mms[c] = []
                        for mi in range(FCHUNK // MMF):
                            mm = nc.tensor.matmul(
                                qn[:, bass.ts(mi, MMF)], wt[:],
                                sgall[:, bass.ds(c * FCHUNK + mi * MMF, MMF)],
                                start=False, stop=True,
                                skip_group_check=True)
                            add_dep_helper(mm.ins, dve.ins,
                                           reason="accum after DVE write")
                            mms[c].append(mm.ins)
                        q[c] = qn
                # --- DVE-only chunk: cols [LCHUNK : 2*LCHUNK), 2048 wide ---
                if t == 0:
                    m_t = xsl(0, LCHUNK, LCHUNK)
                else:
                    mt = mp.tile([P, LCHUNK], mybir.dt.float32, tag="m")
                    nc.vector.scalar_tensor_tensor(
                        mt[:], neg_mem[:], -0.5, xsl(t, LCHUNK, LCHUNK),
                        _ALU.mult, _ALU.add)
                    m_t = mt[:]
                nc.scalar.activation(sgall[:, bass.ds(LCHUNK, LCHUNK)], m_t,
                                     _F.Sign, bias=biases[0][:])
                if t < T - 1:
                    nm = mp.tile([P, LCHUNK], mybir.dt.float32, tag="nm")
                    nc.vector.scalar_tensor_tensor(
                        nm[:], m_t, 1.0, m_t,
                        _ALU.is_gt, _ALU.subtract)
                    neg_mem = nm
                # coarse store of the whole step's spikes
                eng = nc.sync if t % 2 == 0 else nc.gpsimd
                eng.dma_start(o_ap[t], sgall[:])
    nc.compile()
    return nc


_NC_CACHE: dict = {}


def _get_nc():
    if "nc" not in _NC_CACHE:
        _NC_CACHE["nc"] = build_bass()
    return _NC_CACHE["nc"]


def make_in_maps(x: np.ndarray):
    xs = x.reshape(T, B, C, H, W)
    w8 = (-0.25 * np.eye(P, dtype=np.float32)).astype(ml_dtypes.float8_e4m3)
    in_maps = []
    for i in range(NCORES):
        xi = np.ascontiguousarray(xs[:, i * BL:(i + 1) * BL])
        in_maps.append({"x": xi.reshape(T, P, FREE), "w": w8})
    return in_maps


def kernel(x: np.ndarray) -> np.ndarray:
    x = np.asarray(x)
    assert x.shape == (T * B, C, H, W), x.shape
    in_dtype = x.dtype

    nc = _get_nc()
    res = run_bass_kernel_spmd(nc, make_in_maps(x), list(range(NCORES)))

    out = np.empty((T, B, C, H, W), dtype=np.float32)
    for i in range(NCORES):
        raw = np.asarray(res.results[i]["out"]).view(np.uint8)
        raw = raw.reshape(T, BL, C, H, W)
        # sg is {-1, 0, +1} in fp8e4m3; +1.0 encodes as byte 0x38
        out[:, i * BL:(i + 1) * BL] = (raw == 0x38)
    return out.reshape(T * B, C, H, W).astype(in_dtype, copy=False)


# revision 22
# speedup vs baseline: 1.0198x; 1.0198x over previous
"""LIF spiking-neuron kernel for Trainium2 (Bass/Tile), 8-core data-parallel.

Reference semantics (per element, scan over T=8):
    mem = mem * 0.5 + x_t
    s_t = (mem > 1.0) ? 1.0 : 0.0        # forward value of the spike
    mem = mem - s_t

Spikes leave the device as ONE byte per element: the ACT engine writes
sg_t = fp8e4(Sign(mem_t - thr)) in a single op ({-1,0,+1}; no Relu
needed) and the host decodes spikes as (byte == 0x38), the fp8 encoding
of +1.0. This cuts HBM store traffic 4x vs fp32 (16.78 MB in + 4.19 MB
out per core) and puts the input stream, at ~340 GB/s effective, level
with the compute.

The free axis (4096 fp32/partition after folding batch*C*H*W) is cut
into four 1024-wide chunks that all advance together through the T=8
scan, so each x_t tile is consumed in load order. Per global step the
chunks split across two engine pipelines, sized so the Vector engine
(the throughput limiter at ~123 G elem/s) tracks the DMA load rate:

  PE-assisted pair (cols 0:2048, membrane in PSUM, 1 DVE op/step):
      ACT:  sg_t = fp8(Sign(p_t - (1+c_t)))       # also the output tile
      DVE:  q_{t+1} = 0.5 * p_t + x_{t+1}         # stt into a PSUM bank
      PE:   q_{t+1} += (-0.25 I) @ sg_t           # fp8 matmul-accumulate
  where p_t = mem_t + c_t, c_{t+1} = 0.5*c_t + 0.25 (exact dyadic): the
  reset -0.5*s_t = -0.25*sg_t - 0.25 splits into the PE matmul plus a
  constant folded into the threshold. The accumulate onto the
  DVE-written bank is exact but the dep tracker sees two independent
  writers, so the DVE->PE (and PE->reader) edges are added explicitly.
  PSUM fits exactly two such chunks' ping-pong state (8 banks).

  DVE-only pair (cols 2048:4096, carry neg_mem = -mem in SBUF):
      m_t       = stt(neg_mem, -0.5, x_t, mult, add)
      sg_t      = fp8(Sign(m_t - 1))              # ACT, output tile
      neg_mem_t = stt(m_t, 1.0, m_t, is_gt, subtract)   # (m>1) - m

Each x half-tile has exactly one DMA writer (two DMAs into one tile
defeat the subtile dep tracker) and loads/stores alternate between the
SP and Pool DGE queues; DVE, ACT and PE issue no DMAs.

Sharding: batch dim B=32 (dim 1 after temporal expand) split across 8
cores, 4 per core -> per-core [T=8, 128 partitions, 4096 free] fp32.
"""

import numpy as np
import ml_dtypes

import concourse.bass as bass
import concourse.bacc as bacc
import concourse.tile as tile
from concourse import mybir
from concourse.bass_utils import run_bass_kernel_spmd
from concourse.tile_rust import add_dep_helper

T = 8
B = 32
C = 128
H = 32
W = 32
NCORES = 8
BL = B // NCORES              # 4 batch elements per core
N = BL * C * H * W            # 524288 elements per timestep per core
P = 128                       # SBUF partitions
FREE = N // P                 # 4096 fp32 per partition per timestep
FCHUNK = 1024                 # compute chunk width (2 PSUM banks fp32)
LCHUNK = 2048                 # load chunk (per-dma granularity)
MMF = 512                     # matmul moving free dim / PSUM bank width

_ALU = mybir.AluOpType

# Offset-membrane constants: c_0 = 0, c_{t+1} = 0.5*c_t + 0.25 (dyadic).
_CS = [0.0]
for _ in range(T - 1):
    _CS.append(0.5 * _CS[-1] + 0.25)
_THR = [-(1.0 + c) for c in _CS]   # ACT bias per step, PE-assisted pair


def build_bass(free: int = FREE):
    nc = bacc.Bacc("TRN2", target_bir_lowering=False, debug=False,
                   num_devices=NCORES)
    x_ap = nc.dram_tensor("x", [T, P, free], mybir.dt.float32,
                          kind="ExternalInput").ap()
    w_ap = nc.dram_tensor("w", [P, P], mybir.dt.float8e4,
                          kind="ExternalInput").ap()
    o_ap = nc.dram_tensor("out", [T, P, free], mybir.dt.float8e4,
                          kind="ExternalOutput").ap()

    nhalf = free // LCHUNK
    _F = mybir.ActivationFunctionType
    with tile.TileContext(nc) as tc:
        with (
            tc.tile_pool(name="xp", bufs=T) as xp,
            tc.tile_pool(name="xqp", bufs=2) as xqp,
            tc.tile_pool(name="sgp", bufs=6) as sgp,
            tc.tile_pool(name="mp", bufs=2) as mp,
            tc.psum_pool(name="qp", bufs=4) as qp,
            tc.tile_pool(name="cp", bufs=1) as cp,
        ):
            wt = cp.tile([P, P], mybir.dt.float8e4, tag="w")
            nc.gpsimd.dma_start(wt[:], w_ap)
            biases = []
            for t in range(T):
                bt = cp.tile([P, 1], mybir.dt.float32, tag=f"b{t}")
                nc.gpsimd.memset(bt[:], _THR[t])
                biases.append(bt)

            # Preload all of x in consumption order. t=0/1 use quarter
            # tiles for the PE-pair columns so the very first DVE/ACT ops
            # have data ~8us earlier; the rest are half tiles. Each tile
            # has exactly ONE DMA writer (the subtile dep tracker cannot
            # handle two), units alternate between the two DGE queues.
            units = {}   # (t, lo) -> (tile, width)
            qi = 0

            def load_unit(t, lo, width, pool, tag):
                nonlocal qi
                xt = pool.tile([P, width], mybir.dt.float32, tag=tag)
                eng = nc.sync if qi % 2 == 0 else nc.gpsimd
                qi += 1
                eng.dma_start(xt[:], x_ap[t, :, bass.ds(lo, width)])
                units[(t, lo)] = (xt, width)

            # Need order per step t: the PE pair reads x_{t+1}[0:2048] and
            # the DVE chunk reads x_t[2048:4096]; round-robin the units
            # across the rings in that order so neither ring runs ahead.
            load_unit(0, 0, FCHUNK, xqp, "xq0")
            load_unit(0, FCHUNK, FCHUNK, xqp, "xq1")
            load_unit(0, LCHUNK, LCHUNK, xp, "xh1")
            load_unit(1, 0, FCHUNK, xqp, "xq0")
            load_unit(1, FCHUNK, FCHUNK, xqp, "xq1")
            load_unit(1, LCHUNK, LCHUNK, xp, "xh1")
            for t in range(2, T):
                load_unit(t, 0, LCHUNK, xp, "xh0")
                load_unit(t, LCHUNK, LCHUNK, xp, "xh1")

            def xsl(t, lo, width):
                # [128, width] view of cols [lo, lo+width) of x_t
                for (ut, ulo), (xt, uw) in units.items():
                    if ut == t and ulo <= lo and lo + width <= ulo + uw:
                        return xt[:, bass.ds(lo - ulo, width)]
                raise KeyError((t, lo, width))

            pe_chunks = (0, 1)
            q = {c: None for c in pe_chunks}       # PSUM membrane, PE pair
            mms = {c: [] for c in pe_chunks}       # last accum matmuls
            neg_mem = None                         # SBUF carry, DVE chunk

            for t in range(T):
                # One wide spike tile per step: the chunk Sign ops write
                # disjoint slices and a single coarse DMA stores it,
                # keeping the DGE rings dense (32 small stores would gate
                # the ring heads and idle the DMA engines).
                sgall = sgp.tile([P, free], mybir.dt.float8e4, tag="sg")
                # --- PE-assisted pair: cols [0 : 2*FCHUNK) ---
                for c in pe_chunks:
                    p_t = xsl(0, c * FCHUNK, FCHUNK) if t == 0 else q[c][:]
                    sg = sgall[:, bass.ts(c, FCHUNK)]
                    dve = None
                    if t < T - 1:
                        qn = qp.tile([P, FCHUNK], mybir.dt.float32, tag="q")
                        dve = nc.vector.scalar_tensor_tensor(
                            qn[:], p_t, 0.5, xsl(t + 1, c * FCHUNK, FCHUNK),
                            _ALU.mult, _ALU.add)
                        for m in mms[c]:
                            add_dep_helper(dve.ins, m,
                                           reason="q update reads accum q")
                    act = nc.scalar.activation(sg, p_t, _F.Sign,
                                               bias=biases[t][:])
                    for m in mms[c]:
                        add_dep_helper(act.ins, m,
                                       reason="spike reads accum q")
                    if dve is not None:
                        # The matmul below accumulates onto the DVE-written
                        # bank. Its semaphore wait alone can land inside the
                        # DVE->PSUM write-drain window; ordering ACT after the
                        # stt puts the (sg-dependent) matmul >1us behind the
                        # write, closing the race.
                        add_dep_helper(act.ins, dve.ins,
                                       reason="order spike after q write")
                        mms[c] = []
                        for mi in range(FCHUNK // MMF):
                            mm = nc.tensor.matmul(
                                qn[:, bass.ts(mi, MMF)], wt[:],
                                sgall[:, bass.ds(c * FCHUNK + mi * MMF, MMF)],
                                start=False, stop=True,
                                skip_group_check=True)
                            add_dep_helper(mm.ins, dve.ins,
                                           reason="accum after DVE write")
                            mms[c].append(mm.ins)
                        q[c] = qn
                # --- DVE-only chunk: cols [LCHUNK : 2*LCHUNK), 2048 wide ---
                if t == 0:
                    m_t = xsl(0, LCHUNK, LCHUNK)
                else:
                    mt = mp.tile([P, LCHUNK], mybir.dt.float32, tag="m")
                    nc.vector.scalar_tensor_tensor(
                        mt[:], neg_mem[:], -0.5, xsl(t, LCHUNK, LCHUNK),
                        _ALU.mult, _ALU.add)
                    m_t = mt[:]
                nc.scalar.activation(sgall[:, bass.ds(LCHUNK, LCHUNK)], m_t,
                                     _F.Sign, bias=biases[0][:])
                if t < T - 1:
                    nm = mp.tile([P, LCHUNK], mybir.dt.float32, tag="nm")
                    nc.vector.scalar_tensor_tensor(
                        nm[:], m_t, 1.0, m_t,
                        _ALU.is_gt, _ALU.subtract)
                    neg_mem = nm
                # coarse store, split at the half boundary (each half is
                # covered by whole ACT writes) across the two rings
                nc.sync.dma_start(o_ap[t, :, bass.ds(0, LCHUNK)],
                                  sgall[:, bass.ds(0, LCHUNK)])
                nc.gpsimd.dma_start(o_ap[t, :, bass.ds(LCHUNK, LCHUNK)],
                                    sgall[:, bass.ds(LCHUNK, LCHUNK)])
    nc.compile()
    return nc


_NC_CACHE: dict = {}


def _get_nc():
    if "nc" not in _NC_CACHE:
        _NC_CACHE["nc"] = build_bass()
    return _NC_CACHE["nc"]


def make_in_maps(x: np.ndarray):
    xs = x.reshape(T, B, C, H, W)
    w8 = (-0.25 * np.eye(P, dtype=np.float32)).astype(ml_dtypes.float8_e4m3)
    in_maps = []
    for i in range(NCORES):
        xi = np.ascontiguousarray(xs[:, i * BL:(i + 1) * BL])
        in_maps.append({"x": xi.reshape(T, P, FREE), "w": w8})
    return in_maps


def kernel(x: np.ndarray) -> np.ndarray:
    x = np.asarray(x)
    assert x.shape == (T * B, C, H, W), x.shape
    in_dtype = x.dtype

    nc = _get_nc()
    res = run_bass_kernel_spmd(nc, make_in_maps(x), list(range(NCORES)))

    out = np.empty((T, B, C, H, W), dtype=np.float32)
    for i in range(NCORES):
        raw = np.asarray(res.results[i]["out"]).view(np.uint8)
        raw = raw.reshape(T, BL, C, H, W)
        # sg is {-1, 0, +1} in fp8e4m3; +1.0 encodes as byte 0x38
        out[:, i * BL:(i + 1) * BL] = (raw == 0x38)
    return out.reshape(T * B, C, H, W).astype(in_dtype, copy=False)
